# revision 12
# baseline (speedup 1.0000x reference)
"""Trainium2 Bass kernel for causal top-K GNN message passing.

reference semantics (B=4, T=2048, D=1024, K=8):
    scores = x @ x^T per batch, causal (j <= i)
    A[i,j] = 1 iff j among top-8 causal scores of row i
    msg    = (A @ x) / deg
    out    = gelu(mix*x + (1-mix)*msg) * scale       (gain=*, bias=+ general)

Strategy (8 NeuronCores, SPMD single program):
  - core c handles batch b = c % 4; cores 0-3 take row-tiles t = 15-2g
    (slot g = 0..7), cores 4-7 take t = 14-2g.
  - slot g is compiled for causal width W_g = 128*(16-2g) columns; cores 4-7
    use a per-core pair-swapped row-block permutation of the key/value axis so
    their row-tile lands in the last 128 columns of the slot's width. All
    per-core variation lives in the host-prepared input data; the device
    program is identical across cores.
  - This backend executes instructions serially at a roughly flat per-
    instruction cost (matmul ~60-90us, DVE-f32 ~20-49us, ACT ~100-126us,
    small DMA ~15us, cross-engine sync ~50-100us), so the kernel minimizes
    weighted instruction count:
    * scores in ONE fp32 matmul per (k-chunk, 512-col chunk): 160 calls/iter,
      k-outer so the stationary is reused and accumulation chains interleave
      across PSUM banks; the top-8 DVE ops read the PSUM scores directly.
    * top-8 NEIGHBOR INDICES via max8 + max8_index (no thresholding, no 0/1
      adjacency matrix, no transposes, no second matmul): the 8 indices per
      query drive ONE gpsimd dma_gather that pulls all 1024 neighbor rows of
      x straight from HBM, grouped so each query's 8 rows land in its own
      partition. The gather index tile is built with 8 strided 2-byte DMAs
      (position n = r*128 + i lives at idxs[n%16, n//16]) and must be
      replicated into all 8 groups of 16 partitions (HW DGE cores each read
      their own group; CoreSim only reads group 0).
    * msg*deg = sum of the 8 gathered rows = 3 wide DVE adds (pairwise tree).
    * deg is deterministic (min(row+1, 8)), so (1-mix)/deg ships as a host
      precomputed per-partition constant; one 1024-wide scalar_tensor_tensor
      per slot blends msg with mix*gain*x+bias (fp16 out), then a single
      8192-wide Gelu and one output DMA per iteration.
    * rows 0-6 of each batch have fewer than 8 causal candidates; max8_index
      picks masked entries there, so the host overwrites those 28 rows with
      the exact (trivial: msg = causal running mean) fp32 computation. The
      final *scale is also applied on the host after gather.
"""

import sys
import types

try:
    import concourse  # provided by the runtime environment (axon site)
except ImportError:
    sys.path.insert(0, "/opt/trn_rl_repo")

# run_bass_kernel_spmd imports antenv.axon_hooks when BASS_TRACE is set; the
# module is absent in this image, so provide a no-trace stub.
try:
    import antenv.axon_hooks  # noqa: F401
except ImportError:
    _m = types.ModuleType("antenv.axon_hooks")
    _m.get_axon_ntff_profile_hook = lambda: None
    sys.modules["antenv.axon_hooks"] = _m

import numpy as np

import concourse.bacc as bacc
import concourse.tile as tile
import concourse.mybir as mybir
from concourse.bass_utils import run_bass_kernel_spmd

F32 = mybir.dt.float32
F16 = mybir.dt.float16
U16 = mybir.dt.uint16
I16 = mybir.dt.int16
AF = mybir.ActivationFunctionType
ALU = mybir.AluOpType
AX = mybir.AxisListType

B, T, D, K = 4, 2048, 1024, 8
NCORES = 8
SLOTS = 8
NW = [16 - 2 * g for g in range(SLOTS)]  # slot widths in 128-blocks
BIG = np.float32(3e38)

_cache = {}


def _chunks(w):
    """split [0, w) into <=512 pieces"""
    out = []
    j = 0
    while j < w:
        n = min(512, w - j)
        out.append((j, n))
        j += n
    return out


def _build_program(repeat=1):
    nc = bacc.Bacc("TRN2", target_bir_lowering=False, debug=False,
                   num_devices=NCORES)

    # ---- DRAM I/O (per-core shapes; SPMD identical program) ----
    # fp32 x^T, d-chunk major: [:, k*T + j] = x[b, perm(j), 128k+p]
    xt_d = nc.declare_dram_parameter("xt", [128, 8 * T], F32, isOutput=False)
    # fp16 (x*gain) in permuted row order; dma_gather source (stays in HBM)
    xg_d = nc.declare_dram_parameter("xg", [T, D], F16, isOutput=False)
    # mix*gain*x + bias rows, slot major, fp16 (true row order)
    xr_d = nc.declare_dram_parameter("xr", [128, 8 * D], F16, isOutput=False)
    # causal mask bias for the last 256 columns of each slot
    msk_d = nc.declare_dram_parameter("msk", [128, 256], F32, isOutput=False)
    # per-partition constants: col g = (1-mix)/deg(core, slot g, partition)
    sv_d = nc.declare_dram_parameter("sv", [128, 8], F32, isOutput=False)
    out_d = nc.declare_dram_parameter("out", [128, 8 * D], F16, isOutput=True)

    with tile.TileContext(nc) as tc:
        with (
            tc.tile_pool(name="cst", bufs=1) as cst,
            tc.tile_pool(name="sm", bufs=2) as sm,
            tc.tile_pool(name="ixp", bufs=2) as ixp,
            tc.tile_pool(name="gt", bufs=2) as gtp,
            tc.tile_pool(name="gs", bufs=1) as gsp,
            tc.tile_pool(name="bl", bufs=2) as blp,
            tc.tile_pool(name="ob", bufs=1) as obp,
            tc.tile_pool(name="psS", bufs=2, space="PSUM") as psS_p,
        ):
            xt = cst.tile([128, 8 * T], F32, tag="xt")
            xr = cst.tile([128, 8 * D], F16, tag="xr")
            msk = cst.tile([128, 256], F32, tag="msk")
            sv = cst.tile([128, 8], F32, tag="sv")
            nc.sync.dma_start(xt[:], xt_d[:])
            nc.sync.dma_start(xr[:], xr_d[:])
            nc.sync.dma_start(msk[:], msk_d[:])
            nc.sync.dma_start(sv[:], sv_d[:])

            for gi in range(SLOTS * repeat):
                g = gi % SLOTS
                nw = NW[g]
                W = 128 * nw
                cks = _chunks(W)
                # double-buffered scores PSUM (2 x 4 banks): slot g+1's MM1
                # can start while slot g's consumers drain
                psS = psS_p.tile([128, 2048], F32, tag="psS",
                                 name=f"psS{gi}")

                # ---- MM1: causal scores row-tile (128, W), fp32; k-outer so
                # the stationary is reused across the chunk banks ----
                for k in range(8):
                    q = xt[:, k * T + W - 128:k * T + W]
                    for j0, n in cks:
                        nc.tensor.matmul(psS[:, j0:j0 + n], q,
                                         xt[:, k * T + j0:k * T + j0 + n],
                                         start=(k == 0), stop=(k == 7))

                # causal mask on the last 256 columns (in-place on PSUM)
                nc.vector.tensor_tensor(psS[:, W - 256:W], psS[:, W - 256:W],
                                        msk[:], ALU.min)

                # ---- top-8 values + indices (straight from PSUM) ----
                m8 = sm.tile([128, 8], F32, tag="m8", name=f"m8_{gi}")
                nc.vector.max(m8[:], psS[:, :W])
                ix8 = sm.tile([128, 8], U16, tag="ix8", name=f"ix8_{gi}")
                nc.vector.max_index(ix8[:], m8[:], psS[:, :W])

                # ---- gather index tile: position n = r*128 + i lives at
                # idxs[n%16, n//16] = idxs[i%16, 8r + i//16]; build group 0
                # with 8 strided DMAs, replicate to the other 7 groups ----
                idxs = ixp.tile([128, 64], I16, tag="idxs",
                                name=f"idxs{gi}")
                for ib in range(8):
                    nc.sync.dma_start(idxs[0:16, ib:64:8].bitcast(U16),
                                      ix8[16 * ib:16 * ib + 16, :])
                for sz in (16, 32, 64):  # doubling replication: 3 DMAs
                    nc.sync.dma_start(idxs[sz:2 * sz, :], idxs[0:sz, :])

                # ---- gather all 1024 neighbor rows from HBM in one go ----
                gath = gtp.tile([128, 8 * D], F16, tag="gath",
                                name=f"gath{gi}")
                nc.gpsimd.dma_gather(
                    gath[:].rearrange("p (r d) -> p r d", d=D), xg_d[:],
                    idxs[:], num_idxs=1024, num_idxs_reg=1024, elem_size=D)

                # ---- msg*deg: pairwise-tree sum of the 8 gathered rows ----
                s1 = gsp.tile([128, 4 * D], F32, tag="s1", name=f"s1_{gi}")
                nc.vector.tensor_tensor(s1[:], gath[:, :4 * D],
                                        gath[:, 4 * D:], ALU.add)
                s2 = gsp.tile([128, 2 * D], F32, tag="s2", name=f"s2_{gi}")
                nc.vector.tensor_tensor(s2[:], s1[:, :2 * D], s1[:, 2 * D:],
                                        ALU.add)
                msum = gsp.tile([128, D], F32, tag="msum",
                                name=f"msum{gi}")
                nc.vector.tensor_tensor(msum[:], s2[:, :D], s2[:, D:],
                                        ALU.add)

                # ---- blend (deg-divide via precomputed sv), fp16 out ----
                if g == 0:
                    blall = blp.tile([128, 8 * D], F16, tag="blall",
                                     name=f"blall{gi}")
                nc.vector.scalar_tensor_tensor(
                    blall[:, g * D:(g + 1) * D], msum[:], sv[:, g:g + 1],
                    xr[:, g * D:(g + 1) * D], op0=ALU.mult, op1=ALU.add)

                # ---- once per iteration: one wide Gelu + one output DMA ----
                if g == SLOTS - 1:
                    outsb = obp.tile([128, 8 * D], F16, tag="outsb")
                    nc.scalar.activation(outsb[:], blall[:], AF.Gelu)
                    nc.sync.dma_start(out_d[:], outsb[:])

    nc.finalize()
    return nc


def _prep_inputs(x, gain, bias, log_mix, log_scale):
    """Build the 8 per-core input maps."""
    x = np.asarray(x, dtype=np.float32)
    gain = np.asarray(gain, dtype=np.float32)
    bias = np.asarray(bias, dtype=np.float32)
    mix = np.float32(1.0) / (np.float32(1.0) + np.exp(-np.asarray(log_mix, np.float32)))
    scale = np.log1p(np.exp(np.asarray(log_scale, np.float32))).astype(np.float32) + np.float32(0.01)
    one_minus_mix = np.float32(1.0) - mix

    tril = np.tril(np.ones((128, 128), np.bool_))
    tril_bias = np.where(tril, BIG, -BIG).astype(np.float32)
    keep = np.full((128, 128), BIG, np.float32)
    kill = np.full((128, 128), -BIG, np.float32)

    in_maps = []
    meta = []
    for c in range(NCORES):
        b = c % 4
        grp = c // 4
        if grp == 0:
            perm_blocks = np.arange(16)
            tiles = [15 - 2 * g for g in range(SLOTS)]
            msk = np.concatenate([keep, tril_bias], axis=1)
        else:
            perm_blocks = np.arange(16).reshape(8, 2)[:, ::-1].ravel()
            tiles = [14 - 2 * g for g in range(SLOTS)]
            msk = np.concatenate([kill, tril_bias], axis=1)

        # sv[p, g] = (1-mix)/deg, deg = min(global_row+1, 8) is deterministic
        sv = np.empty((128, 8), np.float32)
        for g in range(SLOTS):
            rows = 128 * tiles[g] + np.arange(128)
            deg = np.minimum(rows + 1, 8).astype(np.float32)
            sv[:, g] = one_minus_mix / deg

        perm_rows = (perm_blocks[:, None] * 128 + np.arange(128)[None, :]).ravel()
        xp = x[b][perm_rows]  # (T, D) permuted rows
        # xt: (128, 8*T), chunk k = x^T[128k:128k+128, :]
        xt = np.ascontiguousarray(
            xp.T.reshape(8, 128, T).transpose(1, 0, 2).reshape(128, 8 * T))
        # xg: (T, D) fp16 (x*gain) permuted rows; dma_gather source
        xg = (xp * gain[None, :]).astype(np.float16)
        # xr: (128, 8*D) fp16 slot-major mix*gain*x + bias (true row order)
        xr = np.empty((128, 8 * D), np.float16)
        for g in range(SLOTS):
            r = 128 * tiles[g]
            xr[:, g * D:(g + 1) * D] = ((mix * gain[None, :]) * x[b, r:r + 128, :] + bias[None, :]).astype(np.float16)
        in_maps.append({
            "xt": xt, "xg": xg, "xr": xr, "msk": msk, "sv": sv,
        })
        meta.append((b, tiles, scale))
    return in_maps, meta


def _host_head_rows(x, gain, bias, mix, scale, nrows=7):
    """Exact outputs for rows 0..nrows-1 of each batch (deg < 8 there: ALL
    causal candidates are selected, so msg is the causal running mean)."""
    import math
    erf = np.vectorize(math.erf)
    xh = x[:, :nrows, :].astype(np.float64)  # (B, nrows, D)
    csum = np.cumsum(xh, axis=1)
    deg = np.arange(1, nrows + 1, dtype=np.float64)[None, :, None]
    msg = csum / deg
    blended = mix * xh + (1.0 - mix) * msg
    z = blended * gain[None, None, :].astype(np.float64) + bias[None, None, :]
    g = 0.5 * z * (1.0 + erf(z / np.sqrt(2.0)))
    return (g * scale).astype(np.float32)


def kernel(x, gain, bias, log_mix, log_scale):
    if "nc" not in _cache:
        _cache["nc"] = _build_program()
    nc = _cache["nc"]
    x = np.asarray(x, dtype=np.float32)
    gain = np.asarray(gain, dtype=np.float32)
    bias = np.asarray(bias, dtype=np.float32)
    in_maps, meta = _prep_inputs(x, gain, bias, log_mix, log_scale)
    res = run_bass_kernel_spmd(nc, in_maps, core_ids=list(range(NCORES)))
    y = np.empty((B, T, D), np.float32)
    for c in range(NCORES):
        b, tiles, scale = meta[c]
        o = res.results[c]["out"].astype(np.float32) * scale  # (128, 8*D)
        for g in range(SLOTS):
            r = 128 * tiles[g]
            y[b, r:r + 128, :] = o[:, g * D:(g + 1) * D]
    # rows 0..6 of each batch: fewer than 8 causal candidates; the device's
    # max8_index picks masked entries there, so compute those exactly here
    mix = np.float32(1.0) / (np.float32(1.0) + np.exp(-np.asarray(log_mix, np.float32)))
    scale = np.log1p(np.exp(np.asarray(log_scale, np.float32))).astype(np.float32) + np.float32(0.01)
    y[:, :7, :] = _host_head_rows(x, gain, bias, float(mix), float(scale))
    return y


# revision 13
# speedup vs baseline: 1.1504x; 1.1504x over previous
"""Trainium2 Bass kernel for causal top-K GNN message passing.

reference semantics (B=4, T=2048, D=1024, K=8):
    scores = x @ x^T per batch, causal (j <= i)
    A[i,j] = 1 iff j among top-8 causal scores of row i
    msg    = (A @ x) / deg
    out    = gelu(mix*x + (1-mix)*msg) * scale       (gain=*, bias=+ general)

Strategy (8 NeuronCores, SPMD single program):
  - core c handles batch b = c % 4; cores 0-3 take row-tiles t = 15-2g
    (slot g = 0..7), cores 4-7 take t = 14-2g.
  - slot g is compiled for causal width W_g = 128*(16-2g) columns; cores 4-7
    use a per-core pair-swapped row-block permutation of the key/value axis so
    their row-tile lands in the last 128 columns of the slot's width. All
    per-core variation lives in the host-prepared input data; the device
    program is identical across cores.
  - This backend executes instructions serially at a roughly flat per-
    instruction cost (matmul ~60-90us, DVE-f32 ~20-49us, ACT ~100-126us,
    small DMA ~15us, cross-engine sync ~50-100us), so the kernel minimizes
    weighted instruction count:
    * scores in ONE fp32 matmul per (k-chunk, 512-col chunk): 160 calls/iter,
      k-outer so the stationary is reused and accumulation chains interleave
      across PSUM banks; the top-8 DVE ops read the PSUM scores directly.
    * top-8 NEIGHBOR INDICES via max8 + max8_index (no thresholding, no 0/1
      adjacency matrix, no transposes, no second matmul): the 8 indices per
      query drive ONE gpsimd dma_gather that pulls all 1024 neighbor rows of
      x straight from HBM, grouped so each query's 8 rows land in its own
      partition. The gather index tile is built with 8 strided 2-byte DMAs
      (position n = r*128 + i lives at idxs[n%16, n//16]) and must be
      replicated into all 8 groups of 16 partitions (HW DGE cores each read
      their own group; CoreSim only reads group 0).
    * msg*deg = sum of the 8 gathered rows = 3 wide DVE adds (pairwise tree).
    * deg is deterministic (min(row+1, 8)), so (1-mix)/deg ships as a host
      precomputed per-partition constant; one 1024-wide scalar_tensor_tensor
      per slot blends msg with mix*gain*x+bias (fp16 out), then a single
      8192-wide Gelu and one output DMA per iteration.
    * rows 0-6 of each batch have fewer than 8 causal candidates; max8_index
      picks masked entries there, so the host overwrites those 28 rows with
      the exact (trivial: msg = causal running mean) fp32 computation. The
      final *scale is also applied on the host after gather.
"""

import sys
import types

try:
    import concourse  # provided by the runtime environment (axon site)
except ImportError:
    sys.path.insert(0, "/opt/trn_rl_repo")

# run_bass_kernel_spmd imports antenv.axon_hooks when BASS_TRACE is set; the
# module is absent in this image, so provide a no-trace stub.
try:
    import antenv.axon_hooks  # noqa: F401
except ImportError:
    _m = types.ModuleType("antenv.axon_hooks")
    _m.get_axon_ntff_profile_hook = lambda: None
    sys.modules["antenv.axon_hooks"] = _m

import numpy as np

import concourse.bacc as bacc
import concourse.tile as tile
import concourse.mybir as mybir
from concourse.bass_utils import run_bass_kernel_spmd

F32 = mybir.dt.float32
F16 = mybir.dt.float16
U16 = mybir.dt.uint16
I16 = mybir.dt.int16
AF = mybir.ActivationFunctionType
ALU = mybir.AluOpType
AX = mybir.AxisListType

B, T, D, K = 4, 2048, 1024, 8
NCORES = 8
SLOTS = 8
NW = [16 - 2 * g for g in range(SLOTS)]  # slot widths in 128-blocks
BIG = np.float32(3e38)

_cache = {}


def _chunks(w):
    """split [0, w) into <=512 pieces"""
    out = []
    j = 0
    while j < w:
        n = min(512, w - j)
        out.append((j, n))
        j += n
    return out


def _build_program(repeat=1):
    nc = bacc.Bacc("TRN2", target_bir_lowering=False, debug=False,
                   num_devices=NCORES)

    # ---- DRAM I/O (per-core shapes; SPMD identical program) ----
    # fp32 x^T, d-chunk major: [:, k*T + j] = x[b, perm(j), 128k+p]
    xt_d = nc.declare_dram_parameter("xt", [128, 8 * T], F32, isOutput=False)
    # fp16 (x*gain) in permuted row order; dma_gather source (stays in HBM)
    xg_d = nc.declare_dram_parameter("xg", [T, D], F16, isOutput=False)
    # mix*gain*x + bias rows, slot major, fp16 (true row order)
    xr_d = nc.declare_dram_parameter("xr", [128, 8 * D], F16, isOutput=False)
    # causal mask bias for the last 256 columns of each slot
    msk_d = nc.declare_dram_parameter("msk", [128, 256], F32, isOutput=False)
    # per-partition constants: col g = (1-mix)/deg(core, slot g, partition)
    sv_d = nc.declare_dram_parameter("sv", [128, 8], F32, isOutput=False)
    out_d = nc.declare_dram_parameter("out", [128, 8 * D], F16, isOutput=True)

    with tile.TileContext(nc) as tc:
        with (
            tc.tile_pool(name="cst", bufs=1) as cst,
            tc.tile_pool(name="sm", bufs=1) as sm,
            tc.tile_pool(name="ixp", bufs=1) as ixp,
            tc.tile_pool(name="gt", bufs=1) as gtp,
            tc.tile_pool(name="bl", bufs=1) as blp,
            tc.tile_pool(name="ob", bufs=1) as obp,
            tc.tile_pool(name="psS", bufs=1, space="PSUM") as psS_p,
        ):
            xt = cst.tile([128, 8 * T], F32, tag="xt")
            xr = cst.tile([128, 8 * D], F16, tag="xr")
            msk = cst.tile([128, 256], F32, tag="msk")
            sv = cst.tile([128, 8], F32, tag="sv")
            nc.sync.dma_start(xt[:], xt_d[:])
            nc.sync.dma_start(xr[:], xr_d[:])
            nc.sync.dma_start(msk[:], msk_d[:])
            nc.sync.dma_start(sv[:], sv_d[:])

            psS = psS_p.tile([128, 2048], F32, tag="psS")   # 4 banks

            for gi in range(SLOTS * repeat):
                g = gi % SLOTS
                nw = NW[g]
                W = 128 * nw
                cks = _chunks(W)

                # ---- MM1: causal scores row-tile (128, W), fp32; k-outer so
                # the stationary is reused across the chunk banks ----
                for k in range(8):
                    q = xt[:, k * T + W - 128:k * T + W]
                    for j0, n in cks:
                        nc.tensor.matmul(psS[:, j0:j0 + n], q,
                                         xt[:, k * T + j0:k * T + j0 + n],
                                         start=(k == 0), stop=(k == 7))

                # causal mask on the last 256 columns (in-place on PSUM)
                nc.vector.tensor_tensor(psS[:, W - 256:W], psS[:, W - 256:W],
                                        msk[:], ALU.min)

                # ---- top-8 values + indices (straight from PSUM) ----
                m8 = sm.tile([128, 8], F32, tag="m8")
                nc.vector.max(m8[:], psS[:, :W])
                ix8 = sm.tile([128, 8], U16, tag="ix8")
                nc.vector.max_index(ix8[:], m8[:], psS[:, :W])

                # ---- gather index tile: position n = r*128 + i lives at
                # idxs[n%16, n//16] = idxs[i%16, 8r + i//16]; build group 0
                # with 8 strided DMAs, replicate to the other 7 groups ----
                idxs = ixp.tile([128, 64], I16, tag="idxs")
                for ib in range(8):
                    nc.sync.dma_start(idxs[0:16, ib:64:8].bitcast(U16),
                                      ix8[16 * ib:16 * ib + 16, :])
                for sz in (16, 32, 64):  # doubling replication: 3 DMAs
                    nc.sync.dma_start(idxs[sz:2 * sz, :], idxs[0:sz, :])

                # ---- gather all 1024 neighbor rows from HBM in one go ----
                gath = gtp.tile([128, 8 * D], F16, tag="gath")
                nc.gpsimd.dma_gather(
                    gath[:].rearrange("p (r d) -> p r d", d=D), xg_d[:],
                    idxs[:], num_idxs=1024, num_idxs_reg=1024, elem_size=D)

                # ---- msg*deg: pairwise-tree sum of the 8 gathered rows ----
                s1 = gtp.tile([128, 4 * D], F32, tag="s1")
                nc.vector.tensor_tensor(s1[:], gath[:, :4 * D],
                                        gath[:, 4 * D:], ALU.add)
                s2 = gtp.tile([128, 2 * D], F32, tag="s2")
                nc.vector.tensor_tensor(s2[:], s1[:, :2 * D], s1[:, 2 * D:],
                                        ALU.add)
                msum = gtp.tile([128, D], F32, tag="msum")
                nc.vector.tensor_tensor(msum[:], s2[:, :D], s2[:, D:],
                                        ALU.add)

                # ---- blend (deg-divide via precomputed sv), fp16 out ----
                if g == 0:
                    blall = blp.tile([128, 8 * D], F16, tag="blall",
                                     name=f"blall{gi}")
                nc.vector.scalar_tensor_tensor(
                    blall[:, g * D:(g + 1) * D], msum[:], sv[:, g:g + 1],
                    xr[:, g * D:(g + 1) * D], op0=ALU.mult, op1=ALU.add)

                # ---- once per iteration: one wide Gelu + one output DMA ----
                if g == SLOTS - 1:
                    outsb = obp.tile([128, 8 * D], F16, tag="outsb")
                    nc.scalar.activation(outsb[:], blall[:], AF.Gelu)
                    nc.sync.dma_start(out_d[:], outsb[:])

    nc.finalize()
    return nc


def _prep_inputs(x, gain, bias, log_mix, log_scale):
    """Build the 8 per-core input maps."""
    x = np.asarray(x, dtype=np.float32)
    gain = np.asarray(gain, dtype=np.float32)
    bias = np.asarray(bias, dtype=np.float32)
    mix = np.float32(1.0) / (np.float32(1.0) + np.exp(-np.asarray(log_mix, np.float32)))
    scale = np.log1p(np.exp(np.asarray(log_scale, np.float32))).astype(np.float32) + np.float32(0.01)
    one_minus_mix = np.float32(1.0) - mix

    tril = np.tril(np.ones((128, 128), np.bool_))
    tril_bias = np.where(tril, BIG, -BIG).astype(np.float32)
    keep = np.full((128, 128), BIG, np.float32)
    kill = np.full((128, 128), -BIG, np.float32)

    in_maps = []
    meta = []
    for c in range(NCORES):
        b = c % 4
        grp = c // 4
        if grp == 0:
            perm_blocks = np.arange(16)
            tiles = [15 - 2 * g for g in range(SLOTS)]
            msk = np.concatenate([keep, tril_bias], axis=1)
        else:
            perm_blocks = np.arange(16).reshape(8, 2)[:, ::-1].ravel()
            tiles = [14 - 2 * g for g in range(SLOTS)]
            msk = np.concatenate([kill, tril_bias], axis=1)

        # sv[p, g] = (1-mix)/deg, deg = min(global_row+1, 8) is deterministic
        sv = np.empty((128, 8), np.float32)
        for g in range(SLOTS):
            rows = 128 * tiles[g] + np.arange(128)
            deg = np.minimum(rows + 1, 8).astype(np.float32)
            sv[:, g] = one_minus_mix / deg

        perm_rows = (perm_blocks[:, None] * 128 + np.arange(128)[None, :]).ravel()
        xp = x[b][perm_rows]  # (T, D) permuted rows
        # xt: (128, 8*T), chunk k = x^T[128k:128k+128, :]
        xt = np.ascontiguousarray(
            xp.T.reshape(8, 128, T).transpose(1, 0, 2).reshape(128, 8 * T))
        # xg: (T, D) fp16 (x*gain) permuted rows; dma_gather source
        xg = (xp * gain[None, :]).astype(np.float16)
        # xr: (128, 8*D) fp16 slot-major mix*gain*x + bias (true row order)
        xr = np.empty((128, 8 * D), np.float16)
        for g in range(SLOTS):
            r = 128 * tiles[g]
            xr[:, g * D:(g + 1) * D] = ((mix * gain[None, :]) * x[b, r:r + 128, :] + bias[None, :]).astype(np.float16)
        in_maps.append({
            "xt": xt, "xg": xg, "xr": xr, "msk": msk, "sv": sv,
        })
        meta.append((b, tiles, scale))
    return in_maps, meta


def _host_head_rows(x, gain, bias, mix, scale, nrows=7):
    """Exact outputs for rows 0..nrows-1 of each batch (deg < 8 there: ALL
    causal candidates are selected, so msg is the causal running mean)."""
    import math
    erf = np.vectorize(math.erf)
    xh = x[:, :nrows, :].astype(np.float64)  # (B, nrows, D)
    csum = np.cumsum(xh, axis=1)
    deg = np.arange(1, nrows + 1, dtype=np.float64)[None, :, None]
    msg = csum / deg
    blended = mix * xh + (1.0 - mix) * msg
    z = blended * gain[None, None, :].astype(np.float64) + bias[None, None, :]
    g = 0.5 * z * (1.0 + erf(z / np.sqrt(2.0)))
    return (g * scale).astype(np.float32)


def kernel(x, gain, bias, log_mix, log_scale):
    if "nc" not in _cache:
        _cache["nc"] = _build_program()
    nc = _cache["nc"]
    x = np.asarray(x, dtype=np.float32)
    gain = np.asarray(gain, dtype=np.float32)
    bias = np.asarray(bias, dtype=np.float32)
    in_maps, meta = _prep_inputs(x, gain, bias, log_mix, log_scale)
    res = run_bass_kernel_spmd(nc, in_maps, core_ids=list(range(NCORES)))
    y = np.empty((B, T, D), np.float32)
    for c in range(NCORES):
        b, tiles, scale = meta[c]
        o = res.results[c]["out"].astype(np.float32) * scale  # (128, 8*D)
        for g in range(SLOTS):
            r = 128 * tiles[g]
            y[b, r:r + 128, :] = o[:, g * D:(g + 1) * D]
    # rows 0..6 of each batch: fewer than 8 causal candidates; the device's
    # max8_index picks masked entries there, so compute those exactly here
    mix = np.float32(1.0) / (np.float32(1.0) + np.exp(-np.asarray(log_mix, np.float32)))
    scale = np.log1p(np.exp(np.asarray(log_scale, np.float32))).astype(np.float32) + np.float32(0.01)
    y[:, :7, :] = _host_head_rows(x, gain, bias, float(mix), float(scale))
    return y
